# revision 27
# baseline (speedup 1.0000x reference)
"""Trainium2 Bass kernel for NeuralInelasticModel (3-layer ReLU MLP fwd + analytic Jacobians).

Data-parallel over 8 NeuronCores: each core processes 8192 of the 65536
(ntime*nbatch) samples. Activations are kept feature-major on-chip
(features on SBUF partitions, samples on the free dim) so biases fuse into
the ACT relu and every matmul streams 512-sample tiles at full rate.

The Jacobian J = w3 @ diag(m2) @ w2 @ diag(m1) @ w1 is computed as
  B_i = (w3[i,:] * m2) @ w2   -> 6 dense matmuls with stationary W2i = w2 * w3[i,:,None]
  J_i = (B_i * m1) @ w1       -> M=8 matmuls packed 4-wide into PE column groups
z1 runs as a 3-pass float32r hi/lo split and z2 in true fp32 so the ReLU
masks match the fp32 reference (mask flips near z=0 dominate Jacobian error
otherwise); the Jacobian matmuls run in float32r (fp32 storage, ~13-bit
multiply) for 4x PE rate.
"""

import os
import sys

for _p in ("/root/.axon_site", "/root/.axon_site/_ro/trn_rl_repo",
           "/root/.axon_site/_ro/pypackages", "/opt/trn_rl_repo", "/opt/pypackages"):
    if os.path.isdir(_p) and _p not in sys.path:
        sys.path.append(_p)

import numpy as np

N_CORES = 8
NT, NB = 64, 1024
S = NT * NB
SC = S // N_CORES          # samples per core
NS = 6                     # state size
NI = NS + 2                # input features
H = 256                    # hidden width
TILE = 512                 # samples per on-chip tile (one fp32 PSUM bank)
NTILES = SC // TILE

_PROG = None               # (nc, in_names) cache — build/compile once per process


def _build_program(passes=1, z2_fp32=True):
    """passes>1 repeats the whole computation (same outputs) for timing.
    z2_fp32=True computes z2 with true-fp32 matmuls instead of the
    Veltkamp 3-pass f32r split."""
    import concourse.bacc as bacc
    import concourse.mybir as mybir
    from concourse.bass import ts
    from concourse.tile import TileContext

    f32 = mybir.dt.float32
    f32r = mybir.dt.float32r
    bf16 = mybir.dt.bfloat16
    mult = mybir.AluOpType.mult
    is_gt = mybir.AluOpType.is_gt
    Relu = mybir.ActivationFunctionType.Relu
    VC = 4097.0  # 2**12 + 1: Veltkamp split constant (12-bit hi part)
    Copy = mybir.ActivationFunctionType.Copy

    nc = bacc.Bacc("TRN2", target_bir_lowering=False, debug=False,
                   num_devices=N_CORES)

    xhl_d = nc.dram_tensor("xhl", [NI, 2, SC], f32r, kind="ExternalInput")
    w1Th_d = nc.dram_tensor("w1Th", [NI, H], f32r, kind="ExternalInput")
    w1Tl_d = nc.dram_tensor("w1Tl", [NI, H], f32r, kind="ExternalInput")
    w1c_d = nc.dram_tensor("w1c", [128, 2, NI], f32r, kind="ExternalInput")
    if z2_fp32:
        w2Tc_d = nc.dram_tensor("w2Tc", [128, 2, H], f32, kind="ExternalInput")
    else:
        w2Th_d = nc.dram_tensor("w2Th", [128, 2, H], f32r, kind="ExternalInput")
        w2Tl_d = nc.dram_tensor("w2Tl", [128, 2, H], f32r, kind="ExternalInput")
    w2c_d = nc.dram_tensor("w2c", [128, 2, H], f32, kind="ExternalInput")
    w3Tc_d = nc.dram_tensor("w3Tc", [128, 2, NS], f32r, kind="ExternalInput")
    b1c_d = nc.dram_tensor("b1c", [128, 2], f32, kind="ExternalInput")
    b1Cc_d = nc.dram_tensor("b1Cc", [128, 2], f32, kind="ExternalInput")
    b2c_d = nc.dram_tensor("b2c", [128, 2], f32, kind="ExternalInput")

    out_d = nc.dram_tensor("out", [NS * NI + NS, SC], f32, kind="ExternalOutput")

    with TileContext(nc) as tc:
        with (tc.tile_pool(name="consts", bufs=1) as consts,
              tc.tile_pool(name="acts", bufs=2) as acts,
              tc.tile_pool(name="ypool", bufs=3) as ypool,
              tc.tile_pool(name="psz", bufs=4, space="PSUM") as psz,
              tc.tile_pool(name="psb", bufs=2, space="PSUM") as psb,
              tc.tile_pool(name="pss", bufs=2, space="PSUM") as pss):
            w1Th_sb = consts.tile([NI, H], f32r)
            nc.gpsimd.dma_start(w1Th_sb[:], w1Th_d[:])
            w1Tl_sb = consts.tile([NI, H], f32r)
            nc.gpsimd.dma_start(w1Tl_sb[:], w1Tl_d[:])
            b1_sb = consts.tile([128, 2], f32)
            nc.gpsimd.dma_start(b1_sb[:], b1c_d[:])
            if z2_fp32:
                w2T_sb = consts.tile([128, 2, H], f32)
                nc.gpsimd.dma_start(w2T_sb[:], w2Tc_d[:])
            else:
                w2Th_sb = consts.tile([128, 2, H], f32r)
                nc.gpsimd.dma_start(w2Th_sb[:], w2Th_d[:])
                w2Tl_sb = consts.tile([128, 2, H], f32r)
                nc.gpsimd.dma_start(w2Tl_sb[:], w2Tl_d[:])
            w2_sb = consts.tile([128, 2, H], f32)
            nc.gpsimd.dma_start(w2_sb[:], w2c_d[:])
            w3T_sb = consts.tile([128, 2, NS], f32r)
            nc.gpsimd.dma_start(w3T_sb[:], w3Tc_d[:])
            b2_sb = consts.tile([128, 2], f32)
            nc.gpsimd.dma_start(b2_sb[:], b2c_d[:])
            w1_sb = consts.tile([128, 2, NI], f32r)
            nc.gpsimd.dma_start(w1_sb[:], w1c_d[:])
            b1C_sb = consts.tile([128, 2], f32)
            nc.gpsimd.dma_start(b1C_sb[:], b1Cc_d[:])

            # W2i[:, i, k, :] = w2[k-chunk, :] * w3[i, k-chunk] (per-partition scalar)
            W2i_sb = consts.tile([128, NS, 2, H], f32r)
            for i in range(NS):
                for k in range(2):
                    nc.vector.tensor_scalar(
                        W2i_sb[:, i, k, :], w2_sb[:, k, :],
                        w3T_sb[:, k, i:i + 1].bitcast(f32), None, mult)

            for t in range(NTILES * passes):
                t = t % NTILES
                sl = ts(t, TILE)
                xhl_sb = acts.tile([NI, 2, TILE], f32r, tag="xhl")
                nc.sync.dma_start(xhl_sb[:], xhl_d[:, :, sl])
                xTh_sb = xhl_sb[:, 0, :]
                xTl_sb = xhl_sb[:, 1, :]

                # z1.T = w1 @ x.T as 3-pass f32r hi/lo split (error ~1e-7)
                z1p = [psz.tile([128, TILE], f32, tag="z", name=f"z1p{c}")
                       for c in range(2)]
                for c in range(2):
                    nc.tensor.matmul(z1p[c][:], lhsT=w1Th_sb[:, ts(c, 128)],
                                     rhs=xTh_sb, start=True, stop=False)
                    nc.tensor.matmul(z1p[c][:], lhsT=w1Tl_sb[:, ts(c, 128)],
                                     rhs=xTh_sb, start=False, stop=False)
                    nc.tensor.matmul(z1p[c][:], lhsT=w1Th_sb[:, ts(c, 128)],
                                     rhs=xTl_sb, start=False, stop=True)
                v1_sb = acts.tile([128, 2, TILE], f32, tag="v1")
                t1_sb = (acts.tile([128, 2, TILE], f32, tag="t1", name="t1_sb")
                         if not z2_fp32 else None)
                m1_sb = acts.tile([128, 2, TILE], f32, tag="m1")
                for c in range(2):
                    nc.scalar.activation(v1_sb[:, c, :], z1p[c][:], Relu,
                                         bias=b1_sb[:, c:c + 1])
                    if not z2_fp32:
                        nc.scalar.activation(t1_sb[:, c, :], z1p[c][:],
                                             Relu, bias=b1C_sb[:, c:c + 1],
                                             scale=VC)
                for c in range(2):
                    nc.vector.tensor_scalar(m1_sb[:, c, :], v1_sb[:, c, :],
                                            0.0, None, is_gt)
                if not z2_fp32:
                    # Veltkamp split: v1 = v1h + v1l, v1h 12-bit (FP22-exact)
                    u1_sb = acts.tile([128, 2, TILE], f32, tag="u1")
                    nc.gpsimd.tensor_tensor(u1_sb[:], t1_sb[:], v1_sb[:],
                                            mybir.AluOpType.subtract)
                    v1h_sb = acts.tile([128, 2, TILE], f32r, tag="v1h")
                    nc.gpsimd.tensor_tensor(v1h_sb[:], t1_sb[:], u1_sb[:],
                                            mybir.AluOpType.subtract)
                    v1l_sb = acts.tile([128, 2, TILE], f32r, tag="v1l")
                    nc.gpsimd.tensor_tensor(v1l_sb[:], v1_sb[:],
                                            v1h_sb[:].bitcast(f32),
                                            mybir.AluOpType.subtract)

                # z2.T = w2 @ v1.T with error ~1e-7 so the ReLU masks
                # match the fp32 reference
                z2p = [psz.tile([128, TILE], f32, tag="z", name=f"z2p{c}")
                       for c in range(2)]
                for c in range(2):
                    for k in range(2):
                        if z2_fp32:
                            nc.tensor.matmul(z2p[c][:],
                                             lhsT=w2T_sb[:, k, ts(c, 128)],
                                             rhs=v1_sb[:, k, :],
                                             start=(k == 0), stop=(k == 1))
                        else:
                            nc.tensor.matmul(z2p[c][:],
                                             lhsT=w2Th_sb[:, k, ts(c, 128)],
                                             rhs=v1h_sb[:, k, :],
                                             start=(k == 0), stop=False)
                            nc.tensor.matmul(z2p[c][:],
                                             lhsT=w2Tl_sb[:, k, ts(c, 128)],
                                             rhs=v1h_sb[:, k, :],
                                             start=False, stop=False)
                            nc.tensor.matmul(z2p[c][:],
                                             lhsT=w2Th_sb[:, k, ts(c, 128)],
                                             rhs=v1l_sb[:, k, :],
                                             start=False, stop=(k == 1))
                v2_sb = acts.tile([128, 2, TILE], f32r, tag="v2")
                m2_sb = acts.tile([128, 2, TILE], f32r, tag="m2")
                for c in range(2):
                    nc.scalar.activation(v2_sb[:, c, :], z2p[c][:], Relu,
                                         bias=b2_sb[:, c:c + 1])
                for c in range(2):
                    nc.vector.tensor_scalar(m2_sb[:, c, :], v2_sb[:, c, :],
                                            0.0, None, is_gt)

                # ydot.T = w3 @ v2.T (+ b3 added on host)
                ydp = pss.tile([NI, TILE], f32, tag="sm")
                for k in range(2):
                    nc.tensor.matmul(ydp[:NS, :],
                                     lhsT=w3T_sb[:, k, :],
                                     rhs=v2_sb[:, k, :],
                                     start=(k == 0), stop=(k == 1))
                jall_sb = acts.tile([NI, NS + 1, TILE], f32, tag="jall")
                nc.scalar.activation(jall_sb[:NS, NS, :], ydp[:NS, :], Copy)

                # B_i.T = W2i.T @ m2.T ; Y_i = B_i * m1 ; J_i.T = w1.T @ Y_i.T
                for i in range(NS):
                    yi = ypool.tile([128, 2, TILE], f32r, tag="Y")
                    for c in range(2):
                        bp = psb.tile([128, TILE], f32, tag="bp", name=f"bp{i}_{c}")
                        for k in range(2):
                            nc.tensor.matmul(
                                bp[:],
                                lhsT=W2i_sb[:, i, k, ts(c, 128)],
                                rhs=m2_sb[:, k, :],
                                start=(k == 0), stop=(k == 1))
                        nc.vector.tensor_tensor(yi[:, c, :], bp[:],
                                                m1_sb[:, c, :], mult)
                    jp = pss.tile([NI, TILE], f32, tag="sm")
                    for k in range(2):
                        nc.tensor.matmul(jp[:],
                                         lhsT=w1_sb[:, k, :],
                                         rhs=yi[:, k, :],
                                         start=(k == 0), stop=(k == 1))
                    nc.scalar.activation(jall_sb[:, i, :], jp[:], Copy)
                nc.sync.dma_start(
                    out_d[:NS * NI, sl].rearrange("(i j) s -> j i s", j=NI),
                    jall_sb[:, :NS, :])
                nc.sync.dma_start(out_d[NS * NI:, sl], jall_sb[:NS, NS, :])

    nc.compile()
    import concourse.mybir as _mb
    in_names = []
    for alloc in nc.m.functions[0].allocations:
        if (isinstance(alloc, _mb.MemoryLocationSet)
                and alloc.kind == "ExternalInput"):
            nm = alloc.memorylocations[0].name
            if not nc.partition_id_tensor or nm != nc.partition_id_tensor.name:
                in_names.append(nm)
    return nc, in_names


def _get_program():
    global _PROG
    if _PROG is None:
        _PROG = _build_program()
    return _PROG


def _trunc11(a):
    """Truncate fp32 mantissa to 11 bits (exactly representable in FP22)."""
    u = np.ascontiguousarray(a, dtype=np.float32).view(np.uint32)
    return (u & np.uint32(0xFFFFF000)).view(np.float32)


def _prep_inputs(t, y, erate, T, w1, w2, w3, b1, b2, b3):
    """Host-side layout prep. Returns (in_maps, b3)."""
    f = np.float32
    xT = np.empty((NI, S), dtype=f)
    xT[:NS] = y.reshape(S, NS).T
    xT[NS] = erate.reshape(S)
    xT[NS + 1] = T.reshape(S)
    xhl = np.empty((NI, 2, S), dtype=f)
    xhl[:, 0] = _trunc11(xT)
    xhl[:, 1] = xT - xhl[:, 0]

    def chunked(a):
        # (256, m) -> [128, 2, m] with h = c*128 + p
        return np.ascontiguousarray(
            a.reshape(2, 128, -1).transpose(1, 0, 2)).astype(f, copy=False)

    w1T = np.ascontiguousarray(w1.T, dtype=f)            # (8, 256)
    w1Th = _trunc11(w1T)
    w1Tl = (w1T - w1Th).astype(f)
    w1c = chunked(w1)                                    # [128, 2, 8]
    w2T = np.ascontiguousarray(w2.T)
    w2Th = chunked(_trunc11(w2T))                        # [128, 2, 256]
    w2Tl = chunked((w2T - _trunc11(w2T)).astype(f))
    w2c = chunked(w2)                                    # [128, 2, 256]
    w3Tc = chunked(np.ascontiguousarray(w3.T))           # [128, 2, 6]
    b1c = np.ascontiguousarray(b1.reshape(2, 128).T, dtype=f)   # [128, 2]
    b1Cc = (b1c * np.float32(4097.0)).astype(f)
    b2c = np.ascontiguousarray(b2.reshape(2, 128).T, dtype=f)

    in_maps = []
    for c in range(N_CORES):
        in_maps.append({
            "xhl": np.ascontiguousarray(xhl[:, :, c * SC:(c + 1) * SC]),
            "w1Th": w1Th, "w1Tl": w1Tl, "w1c": w1c, "w2Th": w2Th,
            "w2Tl": w2Tl, "w2Tc": w2Th + w2Tl, "w2c": w2c, "w3Tc": w3Tc,
            "b1c": b1c, "b1Cc": b1Cc, "b2c": b2c,
        })
    return in_maps, np.asarray(b3, dtype=f)


def _assemble(results, b3):
    """Per-core {ydotT, JT} -> full (ydot, dydot_dy, dydot_de, dydot_dT)."""
    f = np.float32
    ydot = np.empty((S, NS), dtype=f)
    J = np.empty((S, NS, NI), dtype=f)
    for c in range(N_CORES):
        sl = slice(c * SC, (c + 1) * SC)
        o = results[c]["out"]
        ydot[sl] = o[NS * NI:].T
        J[sl] = o[:NS * NI].T.reshape(SC, NS, NI)
    ydot += b3
    ydot = ydot.reshape(NT, NB, NS)
    J = J.reshape(NT, NB, NS, NI)
    return (ydot,
            np.ascontiguousarray(J[..., :NS]),
            np.ascontiguousarray(J[..., NS]),
            np.ascontiguousarray(J[..., NS + 1]))


def kernel(t, y, erate, T, w1, w2, w3, b1, b2, b3):
    from concourse.bass_utils import run_bass_kernel_spmd

    nc, in_names = _get_program()
    in_maps, b3 = _prep_inputs(t, y, erate, T, w1, w2, w3, b1, b2, b3)
    in_maps = [{k: m[k] for k in in_names} for m in in_maps]
    res = run_bass_kernel_spmd(nc, in_maps, list(range(N_CORES)))
    return _assemble(res.results, b3)


# revision 29
# speedup vs baseline: 1.3207x; 1.3207x over previous
"""Trainium2 Bass kernel for NeuralInelasticModel (3-layer ReLU MLP fwd + analytic Jacobians).

Data-parallel over 8 NeuronCores: each core processes 8192 of the 65536
(ntime*nbatch) samples. Activations are kept feature-major on-chip
(features on SBUF partitions, samples on the free dim) so biases fuse into
the ACT relu and every matmul streams 512-sample tiles at full rate.

The Jacobian J = w3 @ diag(m2) @ w2 @ diag(m1) @ w1 is computed as
  B_i = (w3[i,:] * m2) @ w2   -> 6 dense matmuls with stationary W2i = w2 * w3[i,:,None]
  J_i = (B_i * m1) @ w1       -> M=8 matmuls packed 4-wide into PE column groups
z1 runs as a 3-pass float32r hi/lo split and z2 in true fp32 so the ReLU
masks match the fp32 reference (mask flips near z=0 dominate Jacobian error
otherwise); the Jacobian matmuls run in float32r (fp32 storage, ~13-bit
multiply) for 4x PE rate.
"""

import os
import sys

for _p in ("/root/.axon_site", "/root/.axon_site/_ro/trn_rl_repo",
           "/root/.axon_site/_ro/pypackages", "/opt/trn_rl_repo", "/opt/pypackages"):
    if os.path.isdir(_p) and _p not in sys.path:
        sys.path.append(_p)

import numpy as np

N_CORES = 8
NT, NB = 64, 1024
S = NT * NB
SC = S // N_CORES          # samples per core
NS = 6                     # state size
NI = NS + 2                # input features
H = 256                    # hidden width
TILE = 512                 # samples per on-chip tile (one fp32 PSUM bank)
NTILES = SC // TILE

_PROG = None               # (nc, in_names) cache — build/compile once per process


def _build_program(passes=1, z2_fp32=True):
    """passes>1 repeats the whole computation (same outputs) for timing.
    z2_fp32=True computes z2 with true-fp32 matmuls instead of the
    Veltkamp 3-pass f32r split."""
    import concourse.bacc as bacc
    import concourse.mybir as mybir
    from concourse.bass import ts
    from concourse.tile import TileContext

    f32 = mybir.dt.float32
    f32r = mybir.dt.float32r
    bf16 = mybir.dt.bfloat16
    mult = mybir.AluOpType.mult
    is_gt = mybir.AluOpType.is_gt
    Relu = mybir.ActivationFunctionType.Relu
    VC = 4097.0  # 2**12 + 1: Veltkamp split constant (12-bit hi part)
    Copy = mybir.ActivationFunctionType.Copy

    nc = bacc.Bacc("TRN2", target_bir_lowering=False, debug=False,
                   num_devices=N_CORES)

    xhl_d = nc.dram_tensor("xhl", [NI, 2, SC], f32r, kind="ExternalInput")
    w1hl_d = nc.dram_tensor("w1hl", [NI, 2, H], f32r, kind="ExternalInput")
    w1c_d = nc.dram_tensor("w1c", [128, 2, NI], f32r, kind="ExternalInput")
    if z2_fp32:
        w2Tc_d = nc.dram_tensor("w2Tc", [128, 2, H], f32, kind="ExternalInput")
    else:
        w2Th_d = nc.dram_tensor("w2Th", [128, 2, H], f32r, kind="ExternalInput")
        w2Tl_d = nc.dram_tensor("w2Tl", [128, 2, H], f32r, kind="ExternalInput")
    w2c_d = nc.dram_tensor("w2c", [128, 2, H], f32, kind="ExternalInput")
    w3Tc_d = nc.dram_tensor("w3Tc", [128, 2, NS], f32r, kind="ExternalInput")
    b1c_d = nc.dram_tensor("b1c", [128, 2], f32, kind="ExternalInput")
    b1Cc_d = nc.dram_tensor("b1Cc", [128, 2], f32, kind="ExternalInput")
    b2c_d = nc.dram_tensor("b2c", [128, 2], f32, kind="ExternalInput")

    out_d = nc.dram_tensor("out", [NS * NI + NS, SC], f32, kind="ExternalOutput")

    with TileContext(nc) as tc:
        with (tc.tile_pool(name="consts", bufs=1) as consts,
              tc.tile_pool(name="acts", bufs=3) as acts,
              tc.tile_pool(name="ypool", bufs=4) as ypool,
              tc.tile_pool(name="psz", bufs=3, space="PSUM") as psz,
              tc.tile_pool(name="psb", bufs=3, space="PSUM") as psb,
              tc.tile_pool(name="pss", bufs=2, space="PSUM") as pss):
            w1hl_sb = consts.tile([NI, 2, H], f32r)
            nc.gpsimd.dma_start(w1hl_sb[:], w1hl_d[:])
            w1Th_sb = w1hl_sb[:, 0, :]
            w1Tl_sb = w1hl_sb[:, 1, :]
            b1_sb = consts.tile([128, 2], f32)
            nc.gpsimd.dma_start(b1_sb[:], b1c_d[:])
            if z2_fp32:
                w2T_sb = consts.tile([128, 2, H], f32)
                nc.gpsimd.dma_start(w2T_sb[:], w2Tc_d[:])
            else:
                w2Th_sb = consts.tile([128, 2, H], f32r)
                nc.gpsimd.dma_start(w2Th_sb[:], w2Th_d[:])
                w2Tl_sb = consts.tile([128, 2, H], f32r)
                nc.gpsimd.dma_start(w2Tl_sb[:], w2Tl_d[:])
            w2_sb = consts.tile([128, 2, H], f32)
            nc.gpsimd.dma_start(w2_sb[:], w2c_d[:])
            w3T_sb = consts.tile([128, 2, NS], f32r)
            nc.gpsimd.dma_start(w3T_sb[:], w3Tc_d[:])
            b2_sb = consts.tile([128, 2], f32)
            nc.gpsimd.dma_start(b2_sb[:], b2c_d[:])
            w1_sb = consts.tile([128, 2, NI], f32r)
            nc.gpsimd.dma_start(w1_sb[:], w1c_d[:])
            b1C_sb = consts.tile([128, 2], f32)
            nc.gpsimd.dma_start(b1C_sb[:], b1Cc_d[:])

            # W2i[:, i, k, :] = w2[k-chunk, :] * w3[i, k-chunk] (per-partition scalar)
            W2i_sb = consts.tile([128, NS, 2, H], f32r)
            for i in range(NS):
                for k in range(2):
                    nc.vector.tensor_scalar(
                        W2i_sb[:, i, k, :], w2_sb[:, k, :],
                        w3T_sb[:, k, i:i + 1].bitcast(f32), None, mult)

            for t in range(NTILES * passes):
                t = t % NTILES
                sl = ts(t, TILE)
                xhl_sb = acts.tile([NI, 2, TILE], f32r, tag="xhl")
                nc.sync.dma_start(xhl_sb[:], xhl_d[:, :, sl])
                xTh_sb = xhl_sb[:, 0, :]
                xTl_sb = xhl_sb[:, 1, :]

                # z1.T = w1 @ x.T as 3-pass f32r hi/lo split (error ~1e-7)
                z1p = [psz.tile([128, TILE], f32, tag="z", name=f"z1p{c}")
                       for c in range(2)]
                for c in range(2):
                    nc.tensor.matmul(z1p[c][:], lhsT=w1Th_sb[:, ts(c, 128)],
                                     rhs=xTh_sb, start=True, stop=False)
                    nc.tensor.matmul(z1p[c][:], lhsT=w1Tl_sb[:, ts(c, 128)],
                                     rhs=xTh_sb, start=False, stop=False)
                    nc.tensor.matmul(z1p[c][:], lhsT=w1Th_sb[:, ts(c, 128)],
                                     rhs=xTl_sb, start=False, stop=True)
                v1_sb = acts.tile([128, 2, TILE], f32, tag="v1")
                t1_sb = (acts.tile([128, 2, TILE], f32, tag="t1", name="t1_sb")
                         if not z2_fp32 else None)
                m1_sb = acts.tile([128, 2, TILE], f32, tag="m1")
                for c in range(2):
                    nc.scalar.activation(v1_sb[:, c, :], z1p[c][:], Relu,
                                         bias=b1_sb[:, c:c + 1])
                    if not z2_fp32:
                        nc.scalar.activation(t1_sb[:, c, :], z1p[c][:],
                                             Relu, bias=b1C_sb[:, c:c + 1],
                                             scale=VC)
                for c in range(2):
                    nc.vector.tensor_scalar(m1_sb[:, c, :], v1_sb[:, c, :],
                                            0.0, None, is_gt)
                if not z2_fp32:
                    # Veltkamp split: v1 = v1h + v1l, v1h 12-bit (FP22-exact)
                    u1_sb = acts.tile([128, 2, TILE], f32, tag="u1")
                    nc.gpsimd.tensor_tensor(u1_sb[:], t1_sb[:], v1_sb[:],
                                            mybir.AluOpType.subtract)
                    v1h_sb = acts.tile([128, 2, TILE], f32r, tag="v1h")
                    nc.gpsimd.tensor_tensor(v1h_sb[:], t1_sb[:], u1_sb[:],
                                            mybir.AluOpType.subtract)
                    v1l_sb = acts.tile([128, 2, TILE], f32r, tag="v1l")
                    nc.gpsimd.tensor_tensor(v1l_sb[:], v1_sb[:],
                                            v1h_sb[:].bitcast(f32),
                                            mybir.AluOpType.subtract)

                # z2.T = w2 @ v1.T with error ~1e-7 so the ReLU masks
                # match the fp32 reference
                z2p = [psz.tile([128, TILE], f32, tag="z", name=f"z2p{c}")
                       for c in range(2)]
                for c in range(2):
                    for k in range(2):
                        if z2_fp32:
                            nc.tensor.matmul(z2p[c][:],
                                             lhsT=w2T_sb[:, k, ts(c, 128)],
                                             rhs=v1_sb[:, k, :],
                                             start=(k == 0), stop=(k == 1))
                        else:
                            nc.tensor.matmul(z2p[c][:],
                                             lhsT=w2Th_sb[:, k, ts(c, 128)],
                                             rhs=v1h_sb[:, k, :],
                                             start=(k == 0), stop=False)
                            nc.tensor.matmul(z2p[c][:],
                                             lhsT=w2Tl_sb[:, k, ts(c, 128)],
                                             rhs=v1h_sb[:, k, :],
                                             start=False, stop=False)
                            nc.tensor.matmul(z2p[c][:],
                                             lhsT=w2Th_sb[:, k, ts(c, 128)],
                                             rhs=v1l_sb[:, k, :],
                                             start=False, stop=(k == 1))
                v2_sb = acts.tile([128, 2, TILE], f32r, tag="v2")
                m2_sb = acts.tile([128, 2, TILE], f32r, tag="m2")
                for c in range(2):
                    nc.scalar.activation(v2_sb[:, c, :], z2p[c][:], Relu,
                                         bias=b2_sb[:, c:c + 1])
                for c in range(2):
                    nc.vector.tensor_scalar(m2_sb[:, c, :], v2_sb[:, c, :],
                                            0.0, None, is_gt)

                # ydot.T = w3 @ v2.T (+ b3 added on host)
                ydp = pss.tile([NI, TILE], f32, tag="sm")
                for k in range(2):
                    nc.tensor.matmul(ydp[:NS, :],
                                     lhsT=w3T_sb[:, k, :],
                                     rhs=v2_sb[:, k, :],
                                     start=(k == 0), stop=(k == 1))
                jall_sb = acts.tile([NI, NS + 1, TILE], f32, tag="jall")
                nc.scalar.activation(jall_sb[:NS, NS, :], ydp[:NS, :], Copy)

                # B_i.T = W2i.T @ m2.T ; Y_i = B_i * m1 ; J_i.T = w1.T @ Y_i.T
                for i in range(NS):
                    yi = ypool.tile([128, 2, TILE], f32r, tag="Y")
                    for c in range(2):
                        bp = psb.tile([128, TILE], f32, tag="bp", name=f"bp{i}_{c}")
                        for k in range(2):
                            nc.tensor.matmul(
                                bp[:],
                                lhsT=W2i_sb[:, i, k, ts(c, 128)],
                                rhs=m2_sb[:, k, :],
                                start=(k == 0), stop=(k == 1))
                        nc.vector.tensor_tensor(yi[:, c, :], bp[:],
                                                m1_sb[:, c, :], mult)
                    jp = pss.tile([NI, TILE], f32, tag="sm")
                    for k in range(2):
                        nc.tensor.matmul(jp[:],
                                         lhsT=w1_sb[:, k, :],
                                         rhs=yi[:, k, :],
                                         start=(k == 0), stop=(k == 1))
                    nc.scalar.activation(jall_sb[:, i, :], jp[:], Copy)
                nc.sync.dma_start(
                    out_d[:NS * NI, sl].rearrange("(i j) s -> j i s", j=NI),
                    jall_sb[:, :NS, :])
                nc.sync.dma_start(out_d[NS * NI:, sl], jall_sb[:NS, NS, :])

    nc.compile()
    import concourse.mybir as _mb
    in_names = []
    for alloc in nc.m.functions[0].allocations:
        if (isinstance(alloc, _mb.MemoryLocationSet)
                and alloc.kind == "ExternalInput"):
            nm = alloc.memorylocations[0].name
            if not nc.partition_id_tensor or nm != nc.partition_id_tensor.name:
                in_names.append(nm)
    return nc, in_names


def _get_program():
    global _PROG
    if _PROG is None:
        _PROG = _build_program()
    return _PROG


def _trunc11(a):
    """Truncate fp32 mantissa to 11 bits (exactly representable in FP22)."""
    u = np.ascontiguousarray(a, dtype=np.float32).view(np.uint32)
    return (u & np.uint32(0xFFFFF000)).view(np.float32)


def _prep_inputs(t, y, erate, T, w1, w2, w3, b1, b2, b3):
    """Host-side layout prep. Returns (in_maps, b3)."""
    f = np.float32
    xT = np.empty((NI, S), dtype=f)
    xT[:NS] = y.reshape(S, NS).T
    xT[NS] = erate.reshape(S)
    xT[NS + 1] = T.reshape(S)
    xhl = np.empty((NI, 2, S), dtype=f)
    xhl[:, 0] = _trunc11(xT)
    xhl[:, 1] = xT - xhl[:, 0]

    def chunked(a):
        # (256, m) -> [128, 2, m] with h = c*128 + p
        return np.ascontiguousarray(
            a.reshape(2, 128, -1).transpose(1, 0, 2)).astype(f, copy=False)

    w1T = np.ascontiguousarray(w1.T, dtype=f)            # (8, 256)
    w1hl = np.empty((NI, 2, H), dtype=f)
    w1hl[:, 0] = _trunc11(w1T)
    w1hl[:, 1] = w1T - w1hl[:, 0]
    w1c = chunked(w1)                                    # [128, 2, 8]
    w2T = np.ascontiguousarray(w2.T)
    w2Th = chunked(_trunc11(w2T))                        # [128, 2, 256]
    w2Tl = chunked((w2T - _trunc11(w2T)).astype(f))
    w2c = chunked(w2)                                    # [128, 2, 256]
    w3Tc = chunked(np.ascontiguousarray(w3.T))           # [128, 2, 6]
    b1c = np.ascontiguousarray(b1.reshape(2, 128).T, dtype=f)   # [128, 2]
    b1Cc = (b1c * np.float32(4097.0)).astype(f)
    b2c = np.ascontiguousarray(b2.reshape(2, 128).T, dtype=f)

    in_maps = []
    for c in range(N_CORES):
        in_maps.append({
            "xhl": np.ascontiguousarray(xhl[:, :, c * SC:(c + 1) * SC]),
            "w1hl": w1hl, "w1c": w1c, "w2Th": w2Th,
            "w2Tl": w2Tl, "w2Tc": w2Th + w2Tl, "w2c": w2c, "w3Tc": w3Tc,
            "b1c": b1c, "b1Cc": b1Cc, "b2c": b2c,
        })
    return in_maps, np.asarray(b3, dtype=f)


def _assemble(results, b3):
    """Per-core {ydotT, JT} -> full (ydot, dydot_dy, dydot_de, dydot_dT)."""
    f = np.float32
    ydot = np.empty((S, NS), dtype=f)
    J = np.empty((S, NS, NI), dtype=f)
    for c in range(N_CORES):
        sl = slice(c * SC, (c + 1) * SC)
        o = results[c]["out"]
        ydot[sl] = o[NS * NI:].T
        J[sl] = o[:NS * NI].T.reshape(SC, NS, NI)
    ydot += b3
    ydot = ydot.reshape(NT, NB, NS)
    J = J.reshape(NT, NB, NS, NI)
    return (ydot,
            np.ascontiguousarray(J[..., :NS]),
            np.ascontiguousarray(J[..., NS]),
            np.ascontiguousarray(J[..., NS + 1]))


def kernel(t, y, erate, T, w1, w2, w3, b1, b2, b3):
    from concourse.bass_utils import run_bass_kernel_spmd

    nc, in_names = _get_program()
    in_maps, b3 = _prep_inputs(t, y, erate, T, w1, w2, w3, b1, b2, b3)
    in_maps = [{k: m[k] for k in in_names} for m in in_maps]
    res = run_bass_kernel_spmd(nc, in_maps, list(range(N_CORES)))
    return _assemble(res.results, b3)


# revision 30
# speedup vs baseline: 1.9411x; 1.4697x over previous
"""Trainium2 Bass kernel for NeuralInelasticModel (3-layer ReLU MLP fwd + analytic Jacobians).

Data-parallel over 8 NeuronCores: each core processes 8192 of the 65536
(ntime*nbatch) samples. Activations are kept feature-major on-chip
(features on SBUF partitions, samples on the free dim) so biases fuse into
the ACT relu and every matmul streams 512-sample tiles at full rate.

The Jacobian J = w3 @ diag(m2) @ w2 @ diag(m1) @ w1 is computed as
  B_i = (w3[i,:] * m2) @ w2   -> 6 dense matmuls with stationary W2i = w2 * w3[i,:,None]
  J_i = (B_i * m1) @ w1       -> M=8 matmuls packed 4-wide into PE column groups
z1 runs as a 3-pass float32r hi/lo split and z2 in true fp32 so the ReLU
masks match the fp32 reference (mask flips near z=0 dominate Jacobian error
otherwise); the Jacobian matmuls run in float32r (fp32 storage, ~13-bit
multiply) for 4x PE rate.
"""

import os
import sys

for _p in ("/root/.axon_site", "/root/.axon_site/_ro/trn_rl_repo",
           "/root/.axon_site/_ro/pypackages", "/opt/trn_rl_repo", "/opt/pypackages"):
    if os.path.isdir(_p) and _p not in sys.path:
        sys.path.append(_p)

import numpy as np

N_CORES = 8
NT, NB = 64, 1024
S = NT * NB
SC = S // N_CORES          # samples per core
NS = 6                     # state size
NI = NS + 2                # input features
H = 256                    # hidden width
TILE = 512                 # samples per on-chip tile (one fp32 PSUM bank)
NTILES = SC // TILE

_PROG = None               # (nc, in_names) cache — build/compile once per process


def _build_program(passes=1, z2_fp32=True):
    """passes>1 repeats the whole computation (same outputs) for timing.
    z2_fp32=True computes z2 with true-fp32 matmuls instead of the
    Veltkamp 3-pass f32r split."""
    import concourse.bacc as bacc
    import concourse.mybir as mybir
    from concourse.bass import ts
    from concourse.tile import TileContext

    f32 = mybir.dt.float32
    f32r = mybir.dt.float32r
    bf16 = mybir.dt.bfloat16
    mult = mybir.AluOpType.mult
    is_gt = mybir.AluOpType.is_gt
    Relu = mybir.ActivationFunctionType.Relu
    VC = 4097.0  # 2**12 + 1: Veltkamp split constant (12-bit hi part)
    Copy = mybir.ActivationFunctionType.Copy

    nc = bacc.Bacc("TRN2", target_bir_lowering=False, debug=False,
                   num_devices=N_CORES)

    x3_d = nc.dram_tensor("x3", [3 * NI, SC], f32r, kind="ExternalInput")
    w1s3_d = nc.dram_tensor("w1s3", [3 * NI, 2, 128], f32r, kind="ExternalInput")
    w1c_d = nc.dram_tensor("w1c", [128, 2, NI], f32r, kind="ExternalInput")
    if z2_fp32:
        w2Tc_d = nc.dram_tensor("w2Tc", [128, 2, H], f32, kind="ExternalInput")
    else:
        w2Th_d = nc.dram_tensor("w2Th", [128, 2, H], f32r, kind="ExternalInput")
        w2Tl_d = nc.dram_tensor("w2Tl", [128, 2, H], f32r, kind="ExternalInput")
    w2c_d = nc.dram_tensor("w2c", [128, 2, H], f32, kind="ExternalInput")
    w3Tc_d = nc.dram_tensor("w3Tc", [128, 2, NS], f32r, kind="ExternalInput")
    b1c_d = nc.dram_tensor("b1c", [128, 2], f32, kind="ExternalInput")
    b1Cc_d = nc.dram_tensor("b1Cc", [128, 2], f32, kind="ExternalInput")
    b2c_d = nc.dram_tensor("b2c", [128, 2], f32, kind="ExternalInput")

    out_d = nc.dram_tensor("out", [NS * NI + NS, SC], f32, kind="ExternalOutput")

    with TileContext(nc) as tc:
        with (tc.tile_pool(name="consts", bufs=1) as consts,
              tc.tile_pool(name="acts", bufs=3) as acts,
              tc.tile_pool(name="ypool", bufs=4) as ypool,
              tc.tile_pool(name="psz", bufs=3, space="PSUM") as psz,
              tc.tile_pool(name="psb", bufs=3, space="PSUM") as psb,
              tc.tile_pool(name="pss", bufs=2, space="PSUM") as pss):
            w1s3_sb = consts.tile([3 * NI, 2, 128], f32r)
            nc.gpsimd.dma_start(w1s3_sb[:], w1s3_d[:])
            b1_sb = consts.tile([128, 2], f32)
            nc.gpsimd.dma_start(b1_sb[:], b1c_d[:])
            if z2_fp32:
                w2T_sb = consts.tile([128, 2, H], f32)
                nc.gpsimd.dma_start(w2T_sb[:], w2Tc_d[:])
            else:
                w2Th_sb = consts.tile([128, 2, H], f32r)
                nc.gpsimd.dma_start(w2Th_sb[:], w2Th_d[:])
                w2Tl_sb = consts.tile([128, 2, H], f32r)
                nc.gpsimd.dma_start(w2Tl_sb[:], w2Tl_d[:])
            w2_sb = consts.tile([128, 2, H], f32)
            nc.gpsimd.dma_start(w2_sb[:], w2c_d[:])
            w3T_sb = consts.tile([128, 2, NS], f32r)
            nc.gpsimd.dma_start(w3T_sb[:], w3Tc_d[:])
            b2_sb = consts.tile([128, 2], f32)
            nc.gpsimd.dma_start(b2_sb[:], b2c_d[:])
            w1_sb = consts.tile([128, 2, NI], f32r)
            nc.gpsimd.dma_start(w1_sb[:], w1c_d[:])
            b1C_sb = consts.tile([128, 2], f32)
            nc.gpsimd.dma_start(b1C_sb[:], b1Cc_d[:])

            # W2i[:, i, k, :] = w2[k-chunk, :] * w3[i, k-chunk] (per-partition scalar)
            W2i_sb = consts.tile([128, NS, 2, H], f32r)
            for i in range(NS):
                for k in range(2):
                    nc.vector.tensor_scalar(
                        W2i_sb[:, i, k, :], w2_sb[:, k, :],
                        w3T_sb[:, k, i:i + 1].bitcast(f32), None, mult)

            for t in range(NTILES * passes):
                t = t % NTILES
                sl = ts(t, TILE)
                x3_sb = acts.tile([3 * NI, TILE], f32r, tag="x3")
                nc.sync.dma_start(x3_sb[:], x3_d[:, sl])

                # z1.T = w1 @ x.T, 3-pass f32r hi/lo split packed into one
                # K=24 matmul: lhsT rows = [w1h; w1l; w1h], rhs = [xh; xh; xl]
                z1p = [psz.tile([128, TILE], f32, tag="z", name=f"z1p{c}")
                       for c in range(2)]
                for c in range(2):
                    nc.tensor.matmul(z1p[c][:], lhsT=w1s3_sb[:, c, :],
                                     rhs=x3_sb[:], start=True, stop=True)
                v1_sb = acts.tile([128, 2, TILE], f32, tag="v1")
                t1_sb = (acts.tile([128, 2, TILE], f32, tag="t1", name="t1_sb")
                         if not z2_fp32 else None)
                m1_sb = acts.tile([128, 2, TILE], f32, tag="m1")
                for c in range(2):
                    nc.scalar.activation(v1_sb[:, c, :], z1p[c][:], Relu,
                                         bias=b1_sb[:, c:c + 1])
                    if not z2_fp32:
                        nc.scalar.activation(t1_sb[:, c, :], z1p[c][:],
                                             Relu, bias=b1C_sb[:, c:c + 1],
                                             scale=VC)
                for c in range(2):
                    nc.vector.tensor_scalar(m1_sb[:, c, :], v1_sb[:, c, :],
                                            0.0, None, is_gt)
                if not z2_fp32:
                    # Veltkamp split: v1 = v1h + v1l, v1h 12-bit (FP22-exact)
                    u1_sb = acts.tile([128, 2, TILE], f32, tag="u1")
                    nc.gpsimd.tensor_tensor(u1_sb[:], t1_sb[:], v1_sb[:],
                                            mybir.AluOpType.subtract)
                    v1h_sb = acts.tile([128, 2, TILE], f32r, tag="v1h")
                    nc.gpsimd.tensor_tensor(v1h_sb[:], t1_sb[:], u1_sb[:],
                                            mybir.AluOpType.subtract)
                    v1l_sb = acts.tile([128, 2, TILE], f32r, tag="v1l")
                    nc.gpsimd.tensor_tensor(v1l_sb[:], v1_sb[:],
                                            v1h_sb[:].bitcast(f32),
                                            mybir.AluOpType.subtract)

                # z2.T = w2 @ v1.T with error ~1e-7 so the ReLU masks
                # match the fp32 reference
                z2p = [psz.tile([128, TILE], f32, tag="z", name=f"z2p{c}")
                       for c in range(2)]
                for c in range(2):
                    for k in range(2):
                        if z2_fp32:
                            nc.tensor.matmul(z2p[c][:],
                                             lhsT=w2T_sb[:, k, ts(c, 128)],
                                             rhs=v1_sb[:, k, :],
                                             start=(k == 0), stop=(k == 1))
                        else:
                            nc.tensor.matmul(z2p[c][:],
                                             lhsT=w2Th_sb[:, k, ts(c, 128)],
                                             rhs=v1h_sb[:, k, :],
                                             start=(k == 0), stop=False)
                            nc.tensor.matmul(z2p[c][:],
                                             lhsT=w2Tl_sb[:, k, ts(c, 128)],
                                             rhs=v1h_sb[:, k, :],
                                             start=False, stop=False)
                            nc.tensor.matmul(z2p[c][:],
                                             lhsT=w2Th_sb[:, k, ts(c, 128)],
                                             rhs=v1l_sb[:, k, :],
                                             start=False, stop=(k == 1))
                v2_sb = acts.tile([128, 2, TILE], f32r, tag="v2")
                m2_sb = acts.tile([128, 2, TILE], f32r, tag="m2")
                for c in range(2):
                    nc.scalar.activation(v2_sb[:, c, :], z2p[c][:], Relu,
                                         bias=b2_sb[:, c:c + 1])
                for c in range(2):
                    nc.vector.tensor_scalar(m2_sb[:, c, :], v2_sb[:, c, :],
                                            0.0, None, is_gt)

                # ydot.T = w3 @ v2.T (+ b3 added on host)
                ydp = pss.tile([NI, TILE], f32, tag="sm")
                for k in range(2):
                    nc.tensor.matmul(ydp[:NS, :],
                                     lhsT=w3T_sb[:, k, :],
                                     rhs=v2_sb[:, k, :],
                                     start=(k == 0), stop=(k == 1))
                jall_sb = acts.tile([NI, NS + 1, TILE], f32, tag="jall")
                nc.scalar.activation(jall_sb[:NS, NS, :], ydp[:NS, :], Copy)

                # B_i.T = W2i.T @ m2.T ; Y_i = B_i * m1 ; J_i.T = w1.T @ Y_i.T
                for i in range(NS):
                    yi = ypool.tile([128, 2, TILE], f32r, tag="Y")
                    for c in range(2):
                        bp = psb.tile([128, TILE], f32, tag="bp", name=f"bp{i}_{c}")
                        for k in range(2):
                            nc.tensor.matmul(
                                bp[:],
                                lhsT=W2i_sb[:, i, k, ts(c, 128)],
                                rhs=m2_sb[:, k, :],
                                start=(k == 0), stop=(k == 1))
                        nc.vector.tensor_tensor(yi[:, c, :], bp[:],
                                                m1_sb[:, c, :], mult)
                    jp = pss.tile([NI, TILE], f32, tag="sm")
                    for k in range(2):
                        nc.tensor.matmul(jp[:],
                                         lhsT=w1_sb[:, k, :],
                                         rhs=yi[:, k, :],
                                         start=(k == 0), stop=(k == 1))
                    nc.scalar.activation(jall_sb[:, i, :], jp[:], Copy)
                nc.sync.dma_start(
                    out_d[:NS * NI, sl].rearrange("(i j) s -> j i s", j=NI),
                    jall_sb[:, :NS, :])
                nc.sync.dma_start(out_d[NS * NI:, sl], jall_sb[:NS, NS, :])

    nc.compile()
    import concourse.mybir as _mb
    in_names = []
    for alloc in nc.m.functions[0].allocations:
        if (isinstance(alloc, _mb.MemoryLocationSet)
                and alloc.kind == "ExternalInput"):
            nm = alloc.memorylocations[0].name
            if not nc.partition_id_tensor or nm != nc.partition_id_tensor.name:
                in_names.append(nm)
    return nc, in_names


def _get_program():
    global _PROG
    if _PROG is None:
        _PROG = _build_program()
    return _PROG


def _trunc11(a):
    """Truncate fp32 mantissa to 11 bits (exactly representable in FP22)."""
    u = np.ascontiguousarray(a, dtype=np.float32).view(np.uint32)
    return (u & np.uint32(0xFFFFF000)).view(np.float32)


def _prep_inputs(t, y, erate, T, w1, w2, w3, b1, b2, b3):
    """Host-side layout prep. Returns (in_maps, b3)."""
    f = np.float32
    xT = np.empty((NI, S), dtype=f)
    xT[:NS] = y.reshape(S, NS).T
    xT[NS] = erate.reshape(S)
    xT[NS + 1] = T.reshape(S)
    x3 = np.empty((3 * NI, S), dtype=f)
    x3[:NI] = _trunc11(xT)
    x3[NI:2 * NI] = x3[:NI]
    x3[2 * NI:] = xT - x3[:NI]

    def chunked(a):
        # (256, m) -> [128, 2, m] with h = c*128 + p
        return np.ascontiguousarray(
            a.reshape(2, 128, -1).transpose(1, 0, 2)).astype(f, copy=False)

    w1T = np.ascontiguousarray(w1.T, dtype=f)            # (8, 256)
    w1Th = _trunc11(w1T)
    w1Tl = (w1T - w1Th).astype(f)
    w1s3 = np.empty((3 * NI, 2, 128), dtype=f)
    for cc in range(2):
        w1s3[:NI, cc] = w1Th[:, cc * 128:(cc + 1) * 128]
        w1s3[NI:2 * NI, cc] = w1Tl[:, cc * 128:(cc + 1) * 128]
        w1s3[2 * NI:, cc] = w1Th[:, cc * 128:(cc + 1) * 128]
    w1c = chunked(w1)                                    # [128, 2, 8]
    w2T = np.ascontiguousarray(w2.T)
    w2Th = chunked(_trunc11(w2T))                        # [128, 2, 256]
    w2Tl = chunked((w2T - _trunc11(w2T)).astype(f))
    w2c = chunked(w2)                                    # [128, 2, 256]
    w3Tc = chunked(np.ascontiguousarray(w3.T))           # [128, 2, 6]
    b1c = np.ascontiguousarray(b1.reshape(2, 128).T, dtype=f)   # [128, 2]
    b1Cc = (b1c * np.float32(4097.0)).astype(f)
    b2c = np.ascontiguousarray(b2.reshape(2, 128).T, dtype=f)

    in_maps = []
    for c in range(N_CORES):
        in_maps.append({
            "x3": np.ascontiguousarray(x3[:, c * SC:(c + 1) * SC]),
            "w1s3": w1s3, "w1c": w1c, "w2Th": w2Th,
            "w2Tl": w2Tl, "w2Tc": w2Th + w2Tl, "w2c": w2c, "w3Tc": w3Tc,
            "b1c": b1c, "b1Cc": b1Cc, "b2c": b2c,
        })
    return in_maps, np.asarray(b3, dtype=f)


def _assemble(results, b3):
    """Per-core {ydotT, JT} -> full (ydot, dydot_dy, dydot_de, dydot_dT)."""
    f = np.float32
    ydot = np.empty((S, NS), dtype=f)
    J = np.empty((S, NS, NI), dtype=f)
    for c in range(N_CORES):
        sl = slice(c * SC, (c + 1) * SC)
        o = results[c]["out"]
        ydot[sl] = o[NS * NI:].T
        J[sl] = o[:NS * NI].T.reshape(SC, NS, NI)
    ydot += b3
    ydot = ydot.reshape(NT, NB, NS)
    J = J.reshape(NT, NB, NS, NI)
    return (ydot,
            np.ascontiguousarray(J[..., :NS]),
            np.ascontiguousarray(J[..., NS]),
            np.ascontiguousarray(J[..., NS + 1]))


def kernel(t, y, erate, T, w1, w2, w3, b1, b2, b3):
    from concourse.bass_utils import run_bass_kernel_spmd

    nc, in_names = _get_program()
    in_maps, b3 = _prep_inputs(t, y, erate, T, w1, w2, w3, b1, b2, b3)
    in_maps = [{k: m[k] for k in in_names} for m in in_maps]
    res = run_bass_kernel_spmd(nc, in_maps, list(range(N_CORES)))
    return _assemble(res.results, b3)
